# revision 10
# baseline (speedup 1.0000x reference)
"""Multi-head cross-attention kernel for Trainium2, 8-way SPMD.

Problem (nn_CrossAttention): B=2, N=2048, DIM=1024, HEADS=16, d=64.
  q = queries @ Wq.T + bq ; k,v likewise
  out = concat_heads(softmax(q_h k_h^T / sqrt(DIM)) v_h)      -> [B, N, DIM]

Sharding: batch x head-group. Core c handles batch c//4, heads
(c%4)*4 .. (c%4)*4+4 (256 feature columns of Wq/Wk/Wv). Each core
computes its heads' projections + full attention locally; host
concatenates the per-core [256, 2048] outputs (feature-major) back to
[B, N, DIM]. No cross-core communication.

Per-core device layout:
  phase 1: qT/kT [256,2048] feature-major (lhsT=WqT-slice, rhs=xT);
           v [2048,256] token-major with an interleaved ones column per
           head ([128,16,4*65] tiles) so the attn@v matmul also emits the
           softmax denominator as output row 64.
  phase 2: per head, per key-tile j: scores S^T[j128, i2048] on PE
           (K=64 contraction) into two [128,1024] PSUM slots (so exp of
           one half overlaps matmuls of the other and the PE never
           stalls long enough for the HAM clock gate to re-throttle),
           exp via ACT (scale=1/32 folded in), P^T @ [v|1] accumulated
           over j into PSUM [65, 2048]. Normalization: reciprocal of
           row 64, broadcast across partitions via a K=1 outer-product
           matmul, one DVE multiply.
All matmuls run as float32r (fp32 bit layout, PE rounds operands on
ingestion, 1 cycle/row). Inputs DMA directly into fp32r tiles.
PSUM budget: SA(2 banks) + SB(2) + AV(4) = 8.
"""

import numpy as np

import concourse.bass as bass
import concourse.mybir as mybir
import concourse.tile as tile
from concourse.bass_utils import run_bass_kernel_spmd

F32 = mybir.dt.float32
F32R = mybir.dt.float32r
AF = mybir.ActivationFunctionType

B, N, DIM, HEADS = 2, 2048, 1024, 16
D = DIM // HEADS          # 64
N_CORES = 8
HPC = HEADS // (N_CORES // B)   # heads per core = 4
FPC = HPC * D                   # feature cols per core = 256
SCALE = DIM ** -0.5
KT = DIM // 128           # k tiles = 8
NT = N // 512             # 512-wide token chunks = 4
JT = N // 128             # key tiles per head = 16


def build_bass(split=True):
    nc = bass.Bass()
    xqT = nc.declare_dram_parameter("xqT", [DIM, N], F32R, isOutput=False)
    xkT = nc.declare_dram_parameter("xkT", [DIM, N], F32R, isOutput=False)
    xvT = nc.declare_dram_parameter("xvT", [DIM, N], F32R, isOutput=False)
    wqT = nc.declare_dram_parameter("wqT", [DIM, FPC], F32R, isOutput=False)
    wkT = nc.declare_dram_parameter("wkT", [DIM, FPC], F32R, isOutput=False)
    wvT = nc.declare_dram_parameter("wvT", [DIM, FPC], F32R, isOutput=False)
    bq = nc.declare_dram_parameter("bq", [2, 128, 1], F32, isOutput=False)
    bk = nc.declare_dram_parameter("bk", [2, 128, 1], F32, isOutput=False)
    bv = nc.declare_dram_parameter("bv", [FPC], F32, isOutput=False)
    outT = nc.declare_dram_parameter("outT", [FPC, N], F32, isOutput=True)

    with tile.TileContext(nc) as tc:
        import contextlib
        with contextlib.ExitStack() as ctx:
            singles = ctx.enter_context(tc.tile_pool(name="singles", bufs=1))
            chunks = ctx.enter_context(tc.tile_pool(name="chunks", bufs=6))
            pts = ctx.enter_context(tc.tile_pool(name="pts", bufs=3))
            avs = ctx.enter_context(tc.tile_pool(name="avs", bufs=2))
            outs = ctx.enter_context(tc.tile_pool(name="outs", bufs=4))
            ps = ctx.enter_context(tc.tile_pool(name="ps", bufs=1, space="PSUM"))

            # --- constants / weights -------------------------------------
            w_r = {}
            for name, dram in (("wq", wqT), ("wk", wkT), ("wv", wvT)):
                w_r[name] = []
                for k in range(KT):
                    wr = singles.tile([128, FPC], F32R, name=f"wr_{name}_{k}",
                                      tag=f"wr_{name}_{k}")
                    nc.sync.dma_start(out=wr, in_=dram[k * 128:(k + 1) * 128, :])
                    w_r[name].append(wr)

            bias_t = {}
            for name, dram in (("bq", bq), ("bk", bk)):
                t = singles.tile([128, 2], F32, name=f"bias_{name}",
                                 tag=f"bias_{name}")
                for m in range(2):
                    nc.sync.dma_start(out=t[:, m:m + 1], in_=dram[m])
                bias_t[name] = t
            bv_b = singles.tile([128, FPC], F32, name="bv_b", tag="bv_b")
            bv_ap = bv[:]
            nc.sync.dma_start(
                out=bv_b,
                in_=bass.AP(tensor=bv_ap.tensor, offset=bv_ap.offset,
                            ap=[[0, 128]] + list(bv_ap.ap)))

            ones_f = singles.tile([128, D], F32, name="ones_f", tag="ones_f")
            nc.vector.memset(ones_f, 1.0)
            ones_r = singles.tile([1, D], F32R, name="ones_r", tag="ones_r")
            nc.vector.tensor_copy(ones_r, ones_f[0:1, :])

            # persistent projection outputs
            qT = [singles.tile([128, N], F32R, name=f"qT_{g}", tag=f"qT_{g}")
                  for g in range(2)]
            kTt = [singles.tile([128, N], F32R, name=f"kT_{g}", tag=f"kT_{g}")
                   for g in range(2)]
            # v with interleaved ones columns: [128 tokens, 16 jtiles, 4*65]
            v_sb = singles.tile([128, JT, HPC * (D + 1)], F32R, name="v_sb",
                                tag="v_sb")
            # ones columns (position D of each head block, every j tile)
            nc.vector.tensor_copy(
                v_sb.rearrange("p j (h e) -> p j h e", h=HPC)[:, :, :, D:D + 1],
                ones_f.rearrange("p (j h e) -> p j h e", j=JT, h=HPC))

            # --- phase 1: projections ------------------------------------
            # Q/K feature-major: out[feat, tok] = W^T-slice.T @ x^T
            for name, xT, dst, bias in (("wq", xqT, qT, "bq"),
                                        ("wk", xkT, kTt, "bk")):
                for n in range(NT):
                    pj = ps.tile([128, 2, 512], F32, name=f"pj_{name}_{n}",
                                 tag="SA" if n % 2 == 0 else "SB")
                    for k in range(KT):
                        ch = chunks.tile([128, 512], F32R,
                                         name=f"ch_{name}_{n}_{k}", tag="ch")
                        nc.sync.dma_start(
                            out=ch,
                            in_=xT[k * 128:(k + 1) * 128,
                                   n * 512:(n + 1) * 512])
                        for m in range(2):
                            nc.tensor.matmul(
                                pj[:, m, :],
                                w_r[name][k][:, m * 128:(m + 1) * 128],
                                ch,
                                start=(k == 0), stop=(k == KT - 1))
                    for m in range(2):
                        nc.vector.tensor_scalar_add(
                            dst[m][:, n * 512:(n + 1) * 512], pj[:, m, :],
                            bias_t[bias][:, m:m + 1])

            # V token-major: out[tok, feat] = x^T-slice.T @ W^T
            for g in range(NT):
                pv = ps.tile([128, 4, 512], F32, name=f"pv_{g}", tag="AV")
                for k in range(KT):
                    ch = chunks.tile([128, 512], F32R, name=f"chv_{g}_{k}",
                                     tag="ch")
                    nc.sync.dma_start(
                        out=ch,
                        in_=xvT[k * 128:(k + 1) * 128, g * 512:(g + 1) * 512])
                    for mt in range(4):
                        nc.tensor.matmul(
                            pv[:, mt, 0:FPC],
                            ch[:, mt * 128:(mt + 1) * 128],
                            w_r["wv"][k],
                            start=(k == 0), stop=(k == KT - 1))
                for mt in range(4):
                    j = g * 4 + mt
                    nc.vector.tensor_add(
                        v_sb[:, j, :].rearrange("p (h e) -> p h e",
                                                h=HPC)[:, :, 0:D],
                        pv[:, mt, 0:FPC].rearrange("p (h d) -> p h d", h=HPC),
                        bv_b.rearrange("p (h d) -> p h d", h=HPC))

            # --- phase 2: attention per head ------------------------------
            # The bc/normalize tail of head h is emitted after head h+1's
            # matmul loop so the in-order PE queue never stalls on the DVE
            # reciprocal chain at a head boundary (which would also let the
            # HAM clock gate re-throttle the PE array).
            def emit_bc_tail(h, av_sb, rec_r):
                for half in range(2):
                    bc_ps = ps.tile([D, 1024], F32, name=f"bc_{h}_{half}",
                                    tag="SA" if half == 0 else "SB")
                    for i2 in range(2):
                        i = half * 2 + i2
                        nc.tensor.matmul(
                            bc_ps[:, i2 * 512:(i2 + 1) * 512],
                            ones_r,
                            rec_r[:, i * 512:(i + 1) * 512],
                            start=True, stop=True)
                    o_sb = outs.tile([D, 1024], F32, name=f"o_{h}_{half}",
                                     tag="o")
                    nc.vector.tensor_mul(
                        o_sb, av_sb[0:D, half * 1024:(half + 1) * 1024], bc_ps)
                    nc.sync.dma_start(
                        out=outT[h * D:(h + 1) * D,
                                 half * 1024:(half + 1) * 1024],
                        in_=o_sb)

            pending_tail = None
            for h in range(HPC):
                g, row = h // 2, (h % 2) * D
                av_ps = ps.tile([D + 1, N], F32, name=f"av_{h}", tag="AV")
                for j in range(JT):
                    kslice = kTt[g][row:row + D, j * 128:(j + 1) * 128]
                    sa = ps.tile([128, 1024], F32, name=f"sa_{h}_{j}", tag="SA")
                    sb = ps.tile([128, 1024], F32, name=f"sb_{h}_{j}", tag="SB")
                    pT = pts.tile([128, N], F32R, name=f"pT_{h}_{j}", tag="pT")
                    for half, sp in ((0, sa), (1, sb)):
                        for i2 in range(2):
                            i = half * 2 + i2
                            nc.tensor.matmul(
                                sp[:, i2 * 512:(i2 + 1) * 512],
                                kslice,
                                qT[g][row:row + D, i * 512:(i + 1) * 512],
                                start=True, stop=True)
                        nc.scalar.activation(
                            pT[:, half * 1024:(half + 1) * 1024], sp,
                            AF.Exp, scale=SCALE)
                    for i in range(NT):
                        nc.tensor.matmul(
                            av_ps[:, i * 512:(i + 1) * 512],
                            v_sb[:, j, h * (D + 1):(h + 1) * (D + 1)],
                            pT[:, i * 512:(i + 1) * 512],
                            start=(j == 0), stop=(j == JT - 1))
                # DVE-only normalization prep; frees the AV psum slot for
                # the next head while its score matmuls are still running.
                av_sb = avs.tile([D + 1, N], F32, name=f"avs_{h}", tag="av")
                nc.vector.tensor_copy(av_sb, av_ps)
                rec = avs.tile([1, N], F32, name=f"rec_{h}", tag="rec")
                nc.vector.reciprocal(rec, av_sb[D:D + 1, :])
                rec_r = avs.tile([1, N], F32R, name=f"recr_{h}", tag="recr")
                nc.vector.tensor_copy(rec_r, rec)
                if pending_tail is not None:
                    emit_bc_tail(*pending_tail)
                pending_tail = (h, av_sb, rec_r)
            emit_bc_tail(*pending_tail)

    if split:
        split_excess_waits(nc)
    return nc


def split_excess_waits(nc, max_waits=1):
    """This walrus codegen accepts one sync wait per instruction; move any
    excess on_wait conditions onto preceding same-engine NoOps."""
    counter = [0]
    for fn in nc.m.functions:
        for blk in fn.blocks:
            new_insts = []
            for inst in blk.instructions:
                si = inst.sync_info
                if si is not None and si.on_wait and len(si.on_wait) > max_waits:
                    waits = list(si.on_wait)
                    excess, keep = waits[:-max_waits], waits[-max_waits:]
                    for w in excess:
                        nop = mybir.InstNoOp(
                            name=f"waitsplit_{counter[0]}", ins=[], outs=[])
                        counter[0] += 1
                        nop.engine = inst.engine
                        nop.sync_info = mybir.SyncInfo(on_wait=[w], on_update=[])
                        new_insts.append(nop)
                    inst.sync_info = mybir.SyncInfo(
                        on_wait=keep, on_update=list(si.on_update or []))
                new_insts.append(inst)
            blk.instructions = new_insts


def make_in_maps(queries, keys, values, Wq, bq, Wk, bk, Wv, bv):
    in_maps = []
    for c in range(N_CORES):
        b = c // (N_CORES // B)
        fs = (c % (N_CORES // B)) * FPC
        fe = fs + FPC
        in_maps.append({
            "xqT": np.ascontiguousarray(queries[b].T),
            "xkT": np.ascontiguousarray(keys[b].T),
            "xvT": np.ascontiguousarray(values[b].T),
            "wqT": np.ascontiguousarray(Wq[fs:fe, :].T),
            "wkT": np.ascontiguousarray(Wk[fs:fe, :].T),
            "wvT": np.ascontiguousarray(Wv[fs:fe, :].T),
            "bq": np.ascontiguousarray(bq[fs:fe]).reshape(2, 128, 1),
            "bk": np.ascontiguousarray(bk[fs:fe]).reshape(2, 128, 1),
            "bv": np.ascontiguousarray(bv[fs:fe]),
        })
    return in_maps


_CACHED_NC = None


def kernel(queries, keys, values, Wq, bq, Wk, bk, Wv, bv):
    global _CACHED_NC
    queries = np.asarray(queries, dtype=np.float32)
    keys = np.asarray(keys, dtype=np.float32)
    values = np.asarray(values, dtype=np.float32)
    Wq = np.asarray(Wq, dtype=np.float32)
    Wk = np.asarray(Wk, dtype=np.float32)
    Wv = np.asarray(Wv, dtype=np.float32)
    bq = np.asarray(bq, dtype=np.float32)
    bk = np.asarray(bk, dtype=np.float32)
    bv = np.asarray(bv, dtype=np.float32)

    if _CACHED_NC is None:
        _CACHED_NC = build_bass()
    nc = _CACHED_NC
    in_maps = make_in_maps(queries, keys, values, Wq, bq, Wk, bk, Wv, bv)
    res = run_bass_kernel_spmd(nc, in_maps, list(range(N_CORES))).results

    out = np.empty((B, N, DIM), dtype=np.float32)
    for c in range(N_CORES):
        b = c // (N_CORES // B)
        fs = (c % (N_CORES // B)) * FPC
        out[b, :, fs:fs + FPC] = res[c]["outT"].T
    return out
